# revision 1
# baseline (speedup 1.0000x reference)
"""Trainium2 Bass kernel for nn_JointNet_23785528885377 (retrieval_knn).

Math note: the reference computes nn_idx = argmin(d2, axis=1) over the full
NxN squared-distance matrix but only consumes row 0 of the gathered
neighbors (exp_neighbor = exp(neigh[0]) = exp(f[nn_idx[0]])). Coords are
ints < 100, so d2 is exact integer arithmetic in fp32, d2[0,0] == 0 is the
global minimum of row 0, and argmin tie-breaks to the lowest index =>
nn_idx[0] == 0 for ANY valid input. Hence exp_neighbor == exp(relu(x[0,:]))
and the whole cdist+argmin is dead code. Per cloud:

    f      = relu(x)                               [N,C]
    rowmax = max_c f                               [N]
    gamma  = max_c(f * exp(f) * exp(-f0)) / rowmax [N]   (f0 = relu(x[0,:]))
    out    = gamma / ||gamma||_2

Sharding: one cloud per core (B=2 clouds; cores 2-7 run the same SPMD
program on duplicate data and are ignored). Everything, including the
final L2 normalization, runs on-device in a single NEFF.
"""

import os

import numpy as np
from contextlib import ExitStack

import concourse.bass as bass
import concourse.bacc as bacc
import concourse.tile as tile
from concourse import mybir
from concourse.bass_utils import run_bass_kernel_spmd

B, N, C = 2, 12288, 32
P = 128
NCORES = 8

AF = mybir.ActivationFunctionType
F32 = mybir.dt.float32


def build_nc(n_rows=N, n_chunks=4, bufs=2, mul2_eng="vector",
             rmax_eng="vector", sq_per_chunk=False,
             skip_compute=False, skip_dma=False):
    """Build + compile the per-core Bass program.

    Inputs : x [n_rows, C] f32 (this core's rows), row0 [1, C] f32 (row 0 of
             this core's cloud). Output: y [n_rows] f32 (normalized gamma).
    Layout: row r lives at partition r // T, free slot r % T (T = n_rows/P),
    so each partition's rows are contiguous in DRAM (T*C*4 bytes per
    partition per DMA descriptor).
    """
    T = n_rows // P
    assert n_rows % (P * n_chunks) == 0
    TCH = T // n_chunks

    nc = bacc.Bacc("TRN2", target_bir_lowering=False, debug=False)
    x = nc.dram_tensor("x", [n_rows, C], F32, kind="ExternalInput")
    r0 = nc.dram_tensor("row0", [1, C], F32, kind="ExternalInput")
    y = nc.dram_tensor("y", [n_rows], F32, kind="ExternalOutput")

    xv = x.rearrange("(p t) c -> p t c", p=P)  # [128, T, 32]
    yv = y.rearrange("(p t) -> p t", p=P)      # [128, T]

    with tile.TileContext(nc) as tc, ExitStack() as ctx:
        pool = ctx.enter_context(tc.tile_pool(name="main", bufs=1))
        ch = ctx.enter_context(tc.tile_pool(name="chunks", bufs=bufs))
        psum = ctx.enter_context(tc.tile_pool(name="psum", bufs=1, space="PSUM"))
        mul2 = getattr(nc, mul2_eng)
        rmax_e = getattr(nc, rmax_eng)

        # e0inv = exp(-relu(row0)) replicated to all 128 partitions via a
        # partition-stride-0 broadcast DMA from DRAM.
        r0rep = pool.tile([P, C], F32)
        r0ap = r0[0, :]
        nc.sync.dma_start(
            out=r0rep[:],
            in_=bass.AP(tensor=r0ap.tensor, offset=r0ap.offset,
                        ap=[[0, P]] + list(r0ap.ap)),
        )
        e0 = pool.tile([P, C], F32)
        nc.scalar.activation(out=e0[:], in_=r0rep[:], func=AF.Relu)
        nc.scalar.activation(out=e0[:], in_=e0[:], func=AF.Exp, scale=-1.0)
        e0ap = e0[:]

        gam = pool.tile([P, T, 1], F32)
        if sq_per_chunk:
            ssq_parts = pool.tile([P, n_chunks], F32)
        for j in range(n_chunks):
            sl = slice(j * TCH, (j + 1) * TCH)
            xt = ch.tile([P, TCH, C], F32, tag="xt")
            if skip_dma:
                nc.vector.memset(xt[:], 0.5)
            else:
                nc.sync.dma_start(out=xt[:], in_=xv[:, sl, :])
            if skip_compute:
                nc.vector.reduce_max(out=gam[:, sl, :], in_=xt[:],
                                     axis=mybir.AxisListType.X)
                continue
            f = ch.tile([P, TCH, C], F32, tag="f")
            nc.scalar.activation(out=f[:], in_=xt[:], func=AF.Relu)
            rmax = ch.tile([P, TCH, 1], F32, tag="rmax")
            rmax_e.reduce_max(out=rmax[:], in_=f[:], axis=mybir.AxisListType.X)
            ex = ch.tile([P, TCH, C], F32, tag="ex")
            nc.scalar.activation(out=ex[:], in_=f[:], func=AF.Exp)
            nc.vector.tensor_mul(ex[:], ex[:], f[:])  # f * exp(f)
            # broadcast e0inv along t via a stride-0 middle dim
            e0b = bass.AP(tensor=e0ap.tensor, offset=e0ap.offset,
                          ap=[e0ap.ap[0], [0, TCH], e0ap.ap[1]])
            mul2.tensor_mul(ex[:], ex[:], e0b)
            m = ch.tile([P, TCH, 1], F32, tag="m")
            nc.vector.reduce_max(out=m[:], in_=ex[:], axis=mybir.AxisListType.X)
            rinv = ch.tile([P, TCH, 1], F32, tag="rinv")
            nc.vector.reciprocal(out=rinv[:], in_=rmax[:])
            nc.vector.tensor_mul(gam[:, sl, :], m[:], rinv[:])
            if sq_per_chunk:
                sqc = ch.tile([P, TCH, 1], F32, tag="sqc")
                nc.scalar.activation(out=sqc[:], in_=gam[:, sl, :],
                                     func=AF.Square,
                                     accum_out=ssq_parts[:, j:j + 1])

        # ||gamma||^2: per-partition sum of squares, then cross-partition sum
        # via PE matmul with a ones vector, broadcast back the same way.
        ssq = pool.tile([P, 1], F32)
        if sq_per_chunk:
            nc.vector.reduce_sum(out=ssq[:], in_=ssq_parts[:],
                                 axis=mybir.AxisListType.X)
        else:
            sq = pool.tile([P, T, 1], F32)
            nc.scalar.activation(out=sq[:], in_=gam[:], func=AF.Square,
                                 accum_out=ssq[:])
        ones = pool.tile([P, 1], F32)
        nc.vector.memset(ones[:], 1.0)
        onesr = pool.tile([1, P], F32)
        nc.vector.memset(onesr[:], 1.0)
        tot = psum.tile([1, 1], F32)
        nc.tensor.matmul(tot[:], ssq[:], ones[:])
        tot_sb = pool.tile([1, 1], F32)
        nc.scalar.activation(out=tot_sb[:], in_=tot[:], func=AF.Copy)
        bc = psum.tile([P, 1], F32)
        nc.tensor.matmul(bc[:], onesr[:], tot_sb[:])
        rec = pool.tile([P, 1], F32)
        nc.vector.reciprocal(out=rec[:], in_=bc[:])
        rstd = pool.tile([P, 1], F32)
        nc.scalar.activation(out=rstd[:], in_=rec[:], func=AF.Sqrt)
        outt = pool.tile([P, T], F32)
        nc.scalar.activation(out=outt[:], in_=gam[:, :, 0], func=AF.Copy,
                             scale=rstd[:])
        nc.sync.dma_start(out=yv[:], in_=outt[:])

    nc.compile()
    return nc


def build_nc2(n_rows=N, n_chunks=8, bufs=4, relu_e="gpsimd", mul1_e="vector",
              mul2_e="gpsimd", gmul_e="gpsimd", scale_e="vector", ablate=()):
    """ACT-lean variant: the scalar engine only runs Exp (+ one Sqrt), so it
    pays the activation-table reload once instead of on every instruction.
    relu/square/scale run as DVE/Pool ALU ops."""
    T = n_rows // P
    assert n_rows % (P * n_chunks) == 0
    TCH = T // n_chunks

    nc = bacc.Bacc("TRN2", target_bir_lowering=False, debug=False)
    x = nc.dram_tensor("x", [n_rows, C], F32, kind="ExternalInput")
    r0 = nc.dram_tensor("row0", [1, C], F32, kind="ExternalInput")
    y = nc.dram_tensor("y", [n_rows], F32, kind="ExternalOutput")

    xv = x.rearrange("(p t) c -> p t c", p=P)
    yv = y.rearrange("(p t) -> p t", p=P)

    with tile.TileContext(nc) as tc, ExitStack() as ctx:
        pool = ctx.enter_context(tc.tile_pool(name="main", bufs=1))
        ch = ctx.enter_context(tc.tile_pool(name="chunks", bufs=bufs))
        psum = ctx.enter_context(tc.tile_pool(name="psum", bufs=1, space="PSUM"))
        relu_eng = getattr(nc, relu_e)
        mul1 = getattr(nc, mul1_e)
        mul2 = getattr(nc, mul2_e)
        gmul = getattr(nc, gmul_e)
        scale_eng = getattr(nc, scale_e)

        r0rep = pool.tile([P, C], F32)
        r0ap = r0[0, :]
        nc.sync.dma_start(
            out=r0rep[:],
            in_=bass.AP(tensor=r0ap.tensor, offset=r0ap.offset,
                        ap=[[0, P]] + list(r0ap.ap)),
        )
        f0 = pool.tile([P, C], F32)
        relu_eng.tensor_scalar_max(f0[:], r0rep[:], 0.0)
        e0 = pool.tile([P, C], F32)
        nc.scalar.activation(out=e0[:], in_=f0[:], func=AF.Exp, scale=-1.0)
        e0ap = e0[:]

        gam = pool.tile([P, T, 1], F32)
        for j in range(n_chunks):
            sl = slice(j * TCH, (j + 1) * TCH)
            xt = ch.tile([P, TCH, C], F32, tag="xt")
            nc.sync.dma_start(out=xt[:], in_=xv[:, sl, :])
            f = ch.tile([P, TCH, C], F32, tag="f")
            if "relu" in ablate:
                f = xt
            else:
                relu_eng.tensor_scalar_max(f[:], xt[:], 0.0)
            rmax = ch.tile([P, TCH, 1], F32, tag="rmax")
            if "red1" in ablate:
                nc.vector.memset(rmax[:], 1.0)
            else:
                nc.vector.reduce_max(out=rmax[:], in_=f[:], axis=mybir.AxisListType.X)
            ex = ch.tile([P, TCH, C], F32, tag="ex")
            if "exp" in ablate:
                ex = f
            else:
                nc.scalar.activation(out=ex[:], in_=f[:], func=AF.Exp)
            if "mul1" not in ablate:
                mul1.tensor_mul(ex[:], ex[:], f[:])
            e0b = bass.AP(tensor=e0ap.tensor, offset=e0ap.offset,
                          ap=[e0ap.ap[0], [0, TCH], e0ap.ap[1]])
            if "mul2" not in ablate:
                mul2.tensor_mul(ex[:], ex[:], e0b)
            m = ch.tile([P, TCH, 1], F32, tag="m")
            if "red2" in ablate:
                nc.vector.memset(m[:], 1.0)
            else:
                nc.vector.reduce_max(out=m[:], in_=ex[:], axis=mybir.AxisListType.X)
            rinv = ch.tile([P, TCH, 1], F32, tag="rinv")
            nc.vector.reciprocal(out=rinv[:], in_=rmax[:])
            gmul.tensor_mul(gam[:, sl, :], m[:], rinv[:])

        gam2d = gam[:, :, 0]
        sq = pool.tile([P, T], F32)
        ssq = pool.tile([P, 1], F32)
        nc.vector.tensor_tensor_reduce(
            out=sq[:], in0=gam2d, in1=gam2d, scale=1.0, scalar=0.0,
            op0=mybir.AluOpType.mult, op1=mybir.AluOpType.add,
            accum_out=ssq[:],
        )
        ones = pool.tile([P, 1], F32)
        nc.vector.memset(ones[:], 1.0)
        onesr = pool.tile([1, P], F32)
        nc.vector.memset(onesr[:], 1.0)
        tot = psum.tile([1, 1], F32)
        nc.tensor.matmul(tot[:], ssq[:], ones[:])
        tot_sb = pool.tile([1, 1], F32)
        nc.vector.tensor_copy(tot_sb[:], tot[:])
        bc = psum.tile([P, 1], F32)
        nc.tensor.matmul(bc[:], onesr[:], tot_sb[:])
        rec = pool.tile([P, 1], F32)
        nc.vector.reciprocal(out=rec[:], in_=bc[:])
        rstd = pool.tile([P, 1], F32)
        nc.scalar.activation(out=rstd[:], in_=rec[:], func=AF.Sqrt)
        outt = pool.tile([P, T], F32)
        scale_eng.tensor_scalar_mul(outt[:], gam2d, rstd[:])
        nc.sync.dma_start(out=yv[:], in_=outt[:])

    nc.compile()
    return nc


def build_nc8(n_chunks=2):
    """All-8-core variant: each core owns a quarter of one cloud's rows.

    Cores 0-3 = cloud 0, cores 4-7 = cloud 1. Each core computes gamma for
    its 3072 rows plus a partial sum(gamma^2); one 4-core AllReduce per
    cloud produces the full norm, then the core scales and writes its slice.
    """
    n_rows = N // 4  # 3072
    T = n_rows // P  # 24
    assert T % n_chunks == 0
    TCH = T // n_chunks

    nc = bacc.Bacc("TRN2", target_bir_lowering=False, debug=False)
    x = nc.dram_tensor("x", [n_rows, C], F32, kind="ExternalInput")
    r0 = nc.dram_tensor("row0", [1, C], F32, kind="ExternalInput")
    y = nc.dram_tensor("y", [n_rows], F32, kind="ExternalOutput")

    xv = x.rearrange("(p t) c -> p t c", p=P)
    yv = y.rearrange("(p t) -> p t", p=P)

    with tile.TileContext(nc) as tc, ExitStack() as ctx:
        pool = ctx.enter_context(tc.tile_pool(name="main", bufs=1))
        ch = ctx.enter_context(tc.tile_pool(name="chunks", bufs=2))
        psum = ctx.enter_context(tc.tile_pool(name="psum", bufs=1, space="PSUM"))
        dram = ctx.enter_context(tc.tile_pool(name="dram", bufs=1, space="DRAM"))

        r0rep = pool.tile([P, C], F32)
        r0ap = r0[0, :]
        nc.sync.dma_start(
            out=r0rep[:],
            in_=bass.AP(tensor=r0ap.tensor, offset=r0ap.offset,
                        ap=[[0, P]] + list(r0ap.ap)),
        )
        e0 = pool.tile([P, C], F32)
        nc.scalar.activation(out=e0[:], in_=r0rep[:], func=AF.Relu)
        nc.scalar.activation(out=e0[:], in_=e0[:], func=AF.Exp, scale=-1.0)
        e0ap = e0[:]

        gam = pool.tile([P, T, 1], F32)
        for j in range(n_chunks):
            sl = slice(j * TCH, (j + 1) * TCH)
            xt = ch.tile([P, TCH, C], F32, tag="xt")
            nc.sync.dma_start(out=xt[:], in_=xv[:, sl, :])
            f = ch.tile([P, TCH, C], F32, tag="f")
            nc.scalar.activation(out=f[:], in_=xt[:], func=AF.Relu)
            rmax = ch.tile([P, TCH, 1], F32, tag="rmax")
            nc.vector.reduce_max(out=rmax[:], in_=f[:], axis=mybir.AxisListType.X)
            ex = ch.tile([P, TCH, C], F32, tag="ex")
            nc.scalar.activation(out=ex[:], in_=f[:], func=AF.Exp)
            nc.vector.tensor_mul(ex[:], ex[:], f[:])
            e0b = bass.AP(tensor=e0ap.tensor, offset=e0ap.offset,
                          ap=[e0ap.ap[0], [0, TCH], e0ap.ap[1]])
            nc.vector.tensor_mul(ex[:], ex[:], e0b)
            m = ch.tile([P, TCH, 1], F32, tag="m")
            nc.vector.reduce_max(out=m[:], in_=ex[:], axis=mybir.AxisListType.X)
            rinv = ch.tile([P, TCH, 1], F32, tag="rinv")
            nc.vector.reciprocal(out=rinv[:], in_=rmax[:])
            nc.vector.tensor_mul(gam[:, sl, :], m[:], rinv[:])

        # partial ||gamma||^2 on this core -> AllReduce within the cloud's
        # 4-core replica group -> total norm on every core.
        sq = pool.tile([P, T, 1], F32)
        ssq = pool.tile([P, 1], F32)
        nc.scalar.activation(out=sq[:], in_=gam[:], func=AF.Square,
                             accum_out=ssq[:])
        ones = pool.tile([P, 1], F32)
        nc.vector.memset(ones[:], 1.0)
        onesr = pool.tile([1, P], F32)
        nc.vector.memset(onesr[:], 1.0)
        tot = psum.tile([1, 1], F32)
        nc.tensor.matmul(tot[:], ssq[:], ones[:])
        tot_sb = pool.tile([1, 1], F32)
        nc.scalar.activation(out=tot_sb[:], in_=tot[:], func=AF.Copy)

        cc_in = dram.tile([1, 1], F32)
        cc_out = dram.tile([1, 1], F32)
        nc.sync.dma_start(out=cc_in[:], in_=tot_sb[:])
        nc.gpsimd.collective_compute(
            "AllReduce",
            mybir.AluOpType.add,
            replica_groups=[[0, 1, 2, 3], [4, 5, 6, 7]],
            ins=[cc_in.opt()],
            outs=[cc_out.opt()],
        )
        tot_all = pool.tile([1, 1], F32)
        nc.sync.dma_start(out=tot_all[:], in_=cc_out[:])

        bc = psum.tile([P, 1], F32)
        nc.tensor.matmul(bc[:], onesr[:], tot_all[:])
        rec = pool.tile([P, 1], F32)
        nc.vector.reciprocal(out=rec[:], in_=bc[:])
        rstd = pool.tile([P, 1], F32)
        nc.scalar.activation(out=rstd[:], in_=rec[:], func=AF.Sqrt)
        outt = pool.tile([P, T], F32)
        nc.scalar.activation(out=outt[:], in_=gam[:, :, 0], func=AF.Copy,
                             scale=rstd[:])
        nc.sync.dma_start(out=yv[:], in_=outt[:])

    nc.compile()
    return nc


_NC_CACHE = {}

IMPL = os.environ.get("KERNEL_IMPL", "v1")


def _get_nc():
    if "nc" not in _NC_CACHE:
        _NC_CACHE["nc"] = build_nc() if IMPL == "v1" else build_nc8()
    return _NC_CACHE["nc"]


def _shard_rows(r):
    """DRAM row r of a shard -> (partition, slot) under the (p t) layout."""
    return r


def make_in_maps(feats):
    in_maps = []
    if IMPL == "v1":
        for core in range(NCORES):
            b = core if core < B else 0  # cores >= B chew duplicate data
            in_maps.append({
                "x": np.ascontiguousarray(feats[b]),
                "row0": np.ascontiguousarray(feats[b, 0:1, :]),
            })
    else:
        q = N // 4
        for core in range(NCORES):
            b, k = divmod(core, 4)
            in_maps.append({
                "x": np.ascontiguousarray(feats[b, k * q:(k + 1) * q]),
                "row0": np.ascontiguousarray(feats[b, 0:1, :]),
            })
    return in_maps


def gather_out(results):
    if IMPL == "v1":
        return np.concatenate([results[b]["y"] for b in range(B)])
    return np.concatenate([results[core]["y"] for core in range(NCORES)])


def kernel(coords: np.ndarray, features: np.ndarray) -> np.ndarray:
    feats = np.ascontiguousarray(np.asarray(features), dtype=np.float32)
    assert feats.shape == (B, N, C), feats.shape
    nc = _get_nc()
    res = run_bass_kernel_spmd(nc, make_in_maps(feats),
                               core_ids=list(range(NCORES)))
    return gather_out(res.results).astype(np.float32)



# revision 5
# speedup vs baseline: 2.6188x; 2.6188x over previous
"""Trainium2 Bass kernel for nn_JointNet_23785528885377 (retrieval_knn).

Math note: the reference computes nn_idx = argmin(d2, axis=1) over the full
NxN squared-distance matrix but only consumes row 0 of the gathered
neighbors (exp_neighbor = exp(neigh[0]) = exp(f[nn_idx[0]])). Coords are
ints < 100, so d2 is exact integer arithmetic in fp32, d2[0,0] == 0 is the
global minimum of row 0, and argmin tie-breaks to the lowest index =>
nn_idx[0] == 0 for ANY valid input. Hence exp_neighbor == exp(relu(x[0,:]))
and the whole cdist+argmin is dead code. Per cloud:

    f      = relu(x)                               [N,C]
    rowmax = max_c f                               [N]
    gamma  = max_c(f * exp(f) * exp(-f0)) / rowmax [N]   (f0 = relu(x[0,:]))
    out    = gamma / ||gamma||_2

Further algebra used by the v3 kernel (valid whenever a row has at least
one nonnegative channel, which holds for this data — all-negative rows
give NaN in the reference anyway via 0/0):

    relu can be dropped inside the max: for x<0 the term x*exp(x)*e0inv
    is negative while the relu'd term is 0, so it never wins the max.
    rowmax = max_c x = ln(max_c exp(x))  (exp monotone), so one dual
    reduce over [exp(x) | x*e0inv*exp(x)] yields both row stats.
    1/sqrt(s) = exp(-0.5*ln(s)) keeps ACT on two tables (Exp, Ln).

Sharding: one cloud per core (B=2 clouds; cores 2-7 run the same SPMD
program on duplicate data and are ignored). A cross-core AllReduce was
measured at ~56us active time on this runtime — far more than the whole
kernel — so the row-sharded 8-core variant loses; data-parallel it is.
"""

import os

import numpy as np
from contextlib import ExitStack

import concourse.bass as bass
import concourse.bacc as bacc
import concourse.tile as tile
from concourse import mybir
from concourse.bass_utils import run_bass_kernel_spmd

B, N, C = 2, 12288, 32
P = 128
NCORES = 8

AF = mybir.ActivationFunctionType
F32 = mybir.dt.float32
F16 = mybir.dt.float16


def build_v3(n_chunks=4, use_f16=True, dual_queue=True):
    """Optimized single-cloud kernel (ACT+DVE+PE only; gpsimd compute and
    tensor_tensor_reduce are broken in this stack — verified empirically).

    Per chunk: DMA xt (alternating HWDGE queues) -> ACT ex=exp(xt) (f16) ->
    DVE t2=xt*e0inv (f16) -> DVE prod=ex*t2 (f16) -> DVE dual reduce_max
    over [P,2,TCH,C] -> [P,2,TCH] (max exp(x) | max x*e0inv*exp(x)) ->
    ACT ln (rowmax via monotonicity) -> DVE reciprocal+mul (gamma).
    Epilogue: ACT Square-accum -> PE sum -> ACT exp(-0.5*ln s) -> PE
    broadcast -> ACT scale -> DMA out. Every chunk gets dedicated tiles
    (bufs=n_chunks): buffer reuse piles multi-engine waits onto the DMA
    trigger and overflows its ISA sync-wait slots.
    """
    T = N // P
    assert T % n_chunks == 0
    TCH = T // n_chunks

    nc = bacc.Bacc("TRN2", target_bir_lowering=False, debug=False)
    x = nc.dram_tensor("x", [N, C], F32, kind="ExternalInput")
    r0 = nc.dram_tensor("row0", [1, C], F32, kind="ExternalInput")
    y = nc.dram_tensor("y", [N], F32, kind="ExternalOutput")

    xv = x.rearrange("(p t) c -> p t c", p=P)  # [128, 96, 32]
    yv = y.rearrange("(p t) -> p t", p=P)      # [128, 96]

    EDT = F16 if use_f16 else F32

    with tile.TileContext(nc) as tc, ExitStack() as ctx:
        pool = ctx.enter_context(tc.tile_pool(name="main", bufs=1))
        ch = ctx.enter_context(tc.tile_pool(name="chunks", bufs=n_chunks))
        psum = ctx.enter_context(tc.tile_pool(name="psum", bufs=1, space="PSUM"))

        # ---- prologue: e0inv = exp(-relu(row0)) broadcast via PE ----
        r0row = pool.tile([1, C], F32)
        nc.sync.dma_start(out=r0row[:], in_=r0[0:1, :])
        f0 = pool.tile([1, C], F32)
        nc.vector.tensor_scalar_max(f0[:], r0row[:], 0.0)
        # Exp table load triggers here, early, overlapping chunk 0's DMA.
        e0row = pool.tile([1, C], F32)
        nc.scalar.activation(out=e0row[:], in_=f0[:], func=AF.Exp, scale=-1.0)
        onesr = pool.tile([1, P], F32)
        nc.vector.memset(onesr[:], 1.0)
        ones = pool.tile([P, 1], F32)
        nc.vector.memset(ones[:], 1.0)
        e0psum = psum.tile([P, C], F32)
        nc.tensor.matmul(e0psum[:], onesr[:], e0row[:])
        e0rep = pool.tile([P, C], F32)
        nc.vector.tensor_copy(e0rep[:], e0psum[:])
        e0ap = e0rep[:]

        mm = pool.tile([P, n_chunks, 2, TCH], F32)
        lnr = pool.tile([P, T], F32)
        gam = pool.tile([P, T, 1], F32)

        for j in range(n_chunks):
            sl = slice(j * TCH, (j + 1) * TCH)
            xt = ch.tile([P, TCH, C], F32, tag=f"xt{j}")
            dmae = nc.scalar if (dual_queue and j % 2 == 1) else nc.sync
            dmae.dma_start(out=xt[:], in_=xv[:, sl, :])

            ep = ch.tile([P, 2, TCH, C], EDT, tag=f"ep{j}")
            nc.scalar.activation(out=ep[:, 0], in_=xt[:], func=AF.Exp)
            t2 = ch.tile([P, TCH, C], EDT, tag=f"t2{j}")
            e0b = bass.AP(tensor=e0ap.tensor, offset=e0ap.offset,
                          ap=[e0ap.ap[0], [0, TCH], e0ap.ap[1]])
            nc.vector.tensor_mul(t2[:], xt[:], e0b)
            nc.vector.tensor_mul(ep[:, 1], ep[:, 0], t2[:])
            nc.vector.reduce_max(out=mm[:, j], in_=ep[:],
                                 axis=mybir.AxisListType.X)
            nc.scalar.activation(out=lnr[:, sl], in_=mm[:, j, 0, :], func=AF.Ln)
            rinv = ch.tile([P, TCH], F32, tag=f"rinv{j}")
            nc.vector.reciprocal(out=rinv[:], in_=lnr[:, sl])
            nc.vector.tensor_mul(gam[:, sl, 0], mm[:, j, 1, :], rinv[:])

        # ---- epilogue: out = gam * exp(-0.5*ln(sum gam^2)) ----
        sq = pool.tile([P, T, 1], F32)
        ssq = pool.tile([P, 1], F32)
        nc.scalar.activation(out=sq[:], in_=gam[:], func=AF.Square,
                             accum_out=ssq[:])
        tot = psum.tile([1, 1], F32)
        nc.tensor.matmul(tot[:], ssq[:], ones[:])
        lntot = pool.tile([1, 1], F32)
        nc.scalar.activation(out=lntot[:], in_=tot[:], func=AF.Ln)
        rstd = pool.tile([1, 1], F32)
        nc.scalar.activation(out=rstd[:], in_=lntot[:], func=AF.Exp, scale=-0.5)
        bc = psum.tile([P, 1], F32)
        nc.tensor.matmul(bc[:], onesr[:], rstd[:])
        bcs = pool.tile([P, 1], F32)
        nc.vector.tensor_copy(bcs[:], bc[:])
        outt = pool.tile([P, T], F32)
        nc.scalar.activation(out=outt[:], in_=gam[:, :, 0], func=AF.Copy,
                             scale=bcs[:])
        nc.sync.dma_start(out=yv[:], in_=outt[:])

    nc.compile()
    return nc


def build_nc(n_rows=N, n_chunks=4, bufs=2):
    """v1 baseline (kept as fallback): relu/exp on ACT, muls+reduces on DVE."""
    T = n_rows // P
    TCH = T // n_chunks

    nc = bacc.Bacc("TRN2", target_bir_lowering=False, debug=False)
    x = nc.dram_tensor("x", [n_rows, C], F32, kind="ExternalInput")
    r0 = nc.dram_tensor("row0", [1, C], F32, kind="ExternalInput")
    y = nc.dram_tensor("y", [n_rows], F32, kind="ExternalOutput")

    xv = x.rearrange("(p t) c -> p t c", p=P)
    yv = y.rearrange("(p t) -> p t", p=P)

    with tile.TileContext(nc) as tc, ExitStack() as ctx:
        pool = ctx.enter_context(tc.tile_pool(name="main", bufs=1))
        ch = ctx.enter_context(tc.tile_pool(name="chunks", bufs=bufs))
        psum = ctx.enter_context(tc.tile_pool(name="psum", bufs=1, space="PSUM"))

        r0rep = pool.tile([P, C], F32)
        r0ap = r0[0, :]
        nc.sync.dma_start(
            out=r0rep[:],
            in_=bass.AP(tensor=r0ap.tensor, offset=r0ap.offset,
                        ap=[[0, P]] + list(r0ap.ap)),
        )
        e0 = pool.tile([P, C], F32)
        nc.scalar.activation(out=e0[:], in_=r0rep[:], func=AF.Relu)
        nc.scalar.activation(out=e0[:], in_=e0[:], func=AF.Exp, scale=-1.0)
        e0ap = e0[:]

        gam = pool.tile([P, T, 1], F32)
        for j in range(n_chunks):
            sl = slice(j * TCH, (j + 1) * TCH)
            xt = ch.tile([P, TCH, C], F32, tag="xt")
            nc.sync.dma_start(out=xt[:], in_=xv[:, sl, :])
            f = ch.tile([P, TCH, C], F32, tag="f")
            nc.scalar.activation(out=f[:], in_=xt[:], func=AF.Relu)
            rmax = ch.tile([P, TCH, 1], F32, tag="rmax")
            nc.vector.reduce_max(out=rmax[:], in_=f[:], axis=mybir.AxisListType.X)
            ex = ch.tile([P, TCH, C], F32, tag="ex")
            nc.scalar.activation(out=ex[:], in_=f[:], func=AF.Exp)
            nc.vector.tensor_mul(ex[:], ex[:], f[:])
            e0b = bass.AP(tensor=e0ap.tensor, offset=e0ap.offset,
                          ap=[e0ap.ap[0], [0, TCH], e0ap.ap[1]])
            nc.vector.tensor_mul(ex[:], ex[:], e0b)
            m = ch.tile([P, TCH, 1], F32, tag="m")
            nc.vector.reduce_max(out=m[:], in_=ex[:], axis=mybir.AxisListType.X)
            rinv = ch.tile([P, TCH, 1], F32, tag="rinv")
            nc.vector.reciprocal(out=rinv[:], in_=rmax[:])
            nc.vector.tensor_mul(gam[:, sl, :], m[:], rinv[:])

        sq = pool.tile([P, T, 1], F32)
        ssq = pool.tile([P, 1], F32)
        nc.scalar.activation(out=sq[:], in_=gam[:], func=AF.Square,
                             accum_out=ssq[:])
        ones = pool.tile([P, 1], F32)
        nc.vector.memset(ones[:], 1.0)
        onesr = pool.tile([1, P], F32)
        nc.vector.memset(onesr[:], 1.0)
        tot = psum.tile([1, 1], F32)
        nc.tensor.matmul(tot[:], ssq[:], ones[:])
        tot_sb = pool.tile([1, 1], F32)
        nc.scalar.activation(out=tot_sb[:], in_=tot[:], func=AF.Copy)
        bc = psum.tile([P, 1], F32)
        nc.tensor.matmul(bc[:], onesr[:], tot_sb[:])
        rec = pool.tile([P, 1], F32)
        nc.vector.reciprocal(out=rec[:], in_=bc[:])
        rstd = pool.tile([P, 1], F32)
        nc.scalar.activation(out=rstd[:], in_=rec[:], func=AF.Sqrt)
        outt = pool.tile([P, T], F32)
        nc.scalar.activation(out=outt[:], in_=gam[:, :, 0], func=AF.Copy,
                             scale=rstd[:])
        nc.sync.dma_start(out=yv[:], in_=outt[:])

    nc.compile()
    return nc


_NC_CACHE = {}

IMPL = os.environ.get("KERNEL_IMPL", "v3")


def _parse_opts():
    """KERNEL_OPTS="n_chunks=6,bufs=2,use_f16=0" -> kwargs for build_v3."""
    opts = {}
    for kv in os.environ.get("KERNEL_OPTS", "").split(","):
        if not kv.strip():
            continue
        k, v = kv.split("=")
        if v in ("0", "1"):
            opts[k.strip()] = bool(int(v))
        elif v.isdigit():
            opts[k.strip()] = int(v)
        else:
            opts[k.strip()] = v.strip()
    return opts


def _get_nc():
    if "nc" not in _NC_CACHE:
        if IMPL == "v1":
            _NC_CACHE["nc"] = build_nc()
        else:
            _NC_CACHE["nc"] = build_v3(**_parse_opts())
    return _NC_CACHE["nc"]


def make_in_maps(feats):
    in_maps = []
    for core in range(NCORES):
        b = core if core < B else 0  # cores >= B chew duplicate data
        in_maps.append({
            "x": np.ascontiguousarray(feats[b]),
            "row0": np.ascontiguousarray(feats[b, 0:1, :]),
        })
    return in_maps


def gather_out(results):
    return np.concatenate([results[b]["y"] for b in range(B)])


def kernel(coords: np.ndarray, features: np.ndarray) -> np.ndarray:
    feats = np.ascontiguousarray(np.asarray(features), dtype=np.float32)
    assert feats.shape == (B, N, C), feats.shape
    nc = _get_nc()
    res = run_bass_kernel_spmd(nc, make_in_maps(feats),
                               core_ids=list(range(NCORES)))
    return gather_out(res.results).astype(np.float32)


# revision 6
# speedup vs baseline: 2.8639x; 1.0936x over previous
"""Trainium2 Bass kernel for nn_JointNet_23785528885377 (retrieval_knn).

Math note: the reference computes nn_idx = argmin(d2, axis=1) over the full
NxN squared-distance matrix but only consumes row 0 of the gathered
neighbors (exp_neighbor = exp(neigh[0]) = exp(f[nn_idx[0]])). Coords are
ints < 100, so d2 is exact integer arithmetic in fp32, d2[0,0] == 0 is the
global minimum of row 0, and argmin tie-breaks to the lowest index =>
nn_idx[0] == 0 for ANY valid input. Hence exp_neighbor == exp(relu(x[0,:]))
and the whole cdist+argmin is dead code. Per cloud:

    f      = relu(x)                               [N,C]
    rowmax = max_c f                               [N]
    gamma  = max_c(f * exp(f) * exp(-f0)) / rowmax [N]   (f0 = relu(x[0,:]))
    out    = gamma / ||gamma||_2

Further algebra used by the v3 kernel (valid whenever a row has at least
one nonnegative channel, which holds for this data — all-negative rows
give NaN in the reference anyway via 0/0):

    relu can be dropped inside the max: for x<0 the term x*exp(x)*e0inv
    is negative while the relu'd term is 0, so it never wins the max.
    rowmax = max_c x = ln(max_c exp(x))  (exp monotone), so one dual
    reduce over [exp(x) | x*e0inv*exp(x)] yields both row stats.
    1/sqrt(s) = exp(-0.5*ln(s)) keeps ACT on two tables (Exp, Ln).

Sharding: one cloud per core (B=2 clouds; cores 2-7 run the same SPMD
program on duplicate data and are ignored). A cross-core AllReduce was
measured at ~56us active time on this runtime — far more than the whole
kernel — so the row-sharded 8-core variant loses; data-parallel it is.
"""

import os

import numpy as np
from contextlib import ExitStack

import concourse.bass as bass
import concourse.bacc as bacc
import concourse.tile as tile
from concourse import mybir
from concourse.bass_utils import run_bass_kernel_spmd

B, N, C = 2, 12288, 32
P = 128
NCORES = 8

AF = mybir.ActivationFunctionType
F32 = mybir.dt.float32
F16 = mybir.dt.float16


def build_v3(n_chunks=4, use_f16=True, dual_queue=True, sq_acc=True):
    """Optimized single-cloud kernel (ACT+DVE+PE only; gpsimd compute and
    tensor_tensor_reduce are broken in this stack — verified empirically).

    Per chunk, only big streaming ops (no cross-engine small-op chains,
    which serialize the pipeline): DMA xt (alternating HWDGE queues) ->
    ACT ex=exp(xt) (f16) -> DVE t2=xt*e0inv (f16) -> DVE prod=ex*t2 (f16)
    -> DVE reduce_max(prod), reduce_max(xt).  All scalar work (1/rowmax,
    gamma, norm) happens once at the end.  A dummy Sqrt right after the
    last exp prefetches the Sqrt activation table off the critical path.
    Every chunk gets dedicated tiles (bufs=n_chunks): buffer reuse piles
    multi-engine waits onto the DMA trigger and overflows its ISA
    sync-wait slots ("Too many sync wait commands").
    """
    T = N // P
    assert T % n_chunks == 0
    TCH = T // n_chunks

    nc = bacc.Bacc("TRN2", target_bir_lowering=False, debug=False)
    x = nc.dram_tensor("x", [N, C], F32, kind="ExternalInput")
    r0 = nc.dram_tensor("row0", [1, C], F32, kind="ExternalInput")
    y = nc.dram_tensor("y", [N], F32, kind="ExternalOutput")

    xv = x.rearrange("(p t) c -> p t c", p=P)  # [128, 96, 32]
    yv = y.rearrange("(p t) -> p t", p=P)      # [128, 96]

    EDT = F16 if use_f16 else F32

    with tile.TileContext(nc) as tc, ExitStack() as ctx:
        pool = ctx.enter_context(tc.tile_pool(name="main", bufs=1))
        ch = ctx.enter_context(tc.tile_pool(name="chunks", bufs=n_chunks))
        psum = ctx.enter_context(tc.tile_pool(name="psum", bufs=1, space="PSUM"))

        # ---- prologue: e0inv = exp(-relu(row0)) broadcast via PE ----
        r0row = pool.tile([1, C], F32)
        nc.sync.dma_start(out=r0row[:], in_=r0[0:1, :])
        f0 = pool.tile([1, C], F32)
        nc.vector.tensor_scalar_max(f0[:], r0row[:], 0.0)
        # Exp table load triggers here, early, overlapping chunk 0's DMA.
        e0row = pool.tile([1, C], F32)
        nc.scalar.activation(out=e0row[:], in_=f0[:], func=AF.Exp, scale=-1.0)
        onesr = pool.tile([1, P], F32)
        nc.vector.memset(onesr[:], 1.0)
        ones = pool.tile([P, 1], F32)
        nc.vector.memset(ones[:], 1.0)
        e0psum = psum.tile([P, C], F32)
        nc.tensor.matmul(e0psum[:], onesr[:], e0row[:])
        e0rep = pool.tile([P, C], F32)
        nc.vector.tensor_copy(e0rep[:], e0psum[:])
        e0ap = e0rep[:]

        # [:, j, 0, :] = rowmax(x), [:, j, 1, :] = rowmax(x*e0inv*exp(x))
        mm = pool.tile([P, n_chunks, 2, TCH], F32)

        for j in range(n_chunks):
            sl = slice(j * TCH, (j + 1) * TCH)
            xt = ch.tile([P, TCH, C], F32, tag=f"xt{j}")
            dmae = nc.scalar if (dual_queue and j % 2 == 1) else nc.sync
            dmae.dma_start(out=xt[:], in_=xv[:, sl, :])

            ex = ch.tile([P, TCH, C], EDT, tag=f"ex{j}")
            nc.scalar.activation(out=ex[:], in_=xt[:], func=AF.Exp)
            t2 = ch.tile([P, TCH, C], EDT, tag=f"t2{j}")
            e0b = bass.AP(tensor=e0ap.tensor, offset=e0ap.offset,
                          ap=[e0ap.ap[0], [0, TCH], e0ap.ap[1]])
            nc.vector.tensor_mul(t2[:], xt[:], e0b)
            prod = ch.tile([P, TCH, C], EDT, tag=f"prod{j}")
            nc.vector.tensor_mul(prod[:], ex[:], t2[:])
            nc.vector.reduce_max(out=mm[:, j, 1, :], in_=prod[:],
                                 axis=mybir.AxisListType.X)
            nc.vector.reduce_max(out=mm[:, j, 0, :], in_=xt[:],
                                 axis=mybir.AxisListType.X)

        # prefetch the Sqrt table while DVE drains the last chunks
        sqdummy = pool.tile([1, 1], F32)
        nc.scalar.activation(out=sqdummy[:], in_=onesr[:, 0:1], func=AF.Sqrt)

        # ---- epilogue ----
        rinv = pool.tile([P, T], F32)
        nc.vector.reciprocal(out=rinv[:], in_=mm[:, :, 0, :])
        gam = pool.tile([P, T], F32)
        nc.vector.tensor_mul(gam[:], mm[:, :, 1, :], rinv[:])
        ssq = pool.tile([P, 1], F32)
        if sq_acc:
            sq = pool.tile([P, T], F32)
            nc.scalar.activation(out=sq[:], in_=gam[:], func=AF.Square,
                                 accum_out=ssq[:])
        else:
            sq = pool.tile([P, T], F32)
            nc.vector.tensor_mul(sq[:], gam[:], gam[:])
            nc.vector.reduce_sum(out=ssq[:], in_=sq[:],
                                 axis=mybir.AxisListType.X)
        tot = psum.tile([1, 1], F32)
        nc.tensor.matmul(tot[:], ssq[:], ones[:])
        rec = pool.tile([1, 1], F32)
        nc.vector.reciprocal(out=rec[:], in_=tot[:])
        rstd = pool.tile([1, 1], F32)
        nc.scalar.activation(out=rstd[:], in_=rec[:], func=AF.Sqrt)
        bc = psum.tile([P, 1], F32)
        nc.tensor.matmul(bc[:], onesr[:], rstd[:])
        bcs = pool.tile([P, 1], F32)
        nc.vector.tensor_copy(bcs[:], bc[:])
        outt = pool.tile([P, T], F32)
        nc.scalar.activation(out=outt[:], in_=gam[:], func=AF.Copy,
                             scale=bcs[:])
        nc.sync.dma_start(out=yv[:], in_=outt[:])

    nc.compile()
    return nc


def build_nc(n_rows=N, n_chunks=4, bufs=2):
    """v1 baseline (kept as fallback): relu/exp on ACT, muls+reduces on DVE."""
    T = n_rows // P
    TCH = T // n_chunks

    nc = bacc.Bacc("TRN2", target_bir_lowering=False, debug=False)
    x = nc.dram_tensor("x", [n_rows, C], F32, kind="ExternalInput")
    r0 = nc.dram_tensor("row0", [1, C], F32, kind="ExternalInput")
    y = nc.dram_tensor("y", [n_rows], F32, kind="ExternalOutput")

    xv = x.rearrange("(p t) c -> p t c", p=P)
    yv = y.rearrange("(p t) -> p t", p=P)

    with tile.TileContext(nc) as tc, ExitStack() as ctx:
        pool = ctx.enter_context(tc.tile_pool(name="main", bufs=1))
        ch = ctx.enter_context(tc.tile_pool(name="chunks", bufs=bufs))
        psum = ctx.enter_context(tc.tile_pool(name="psum", bufs=1, space="PSUM"))

        r0rep = pool.tile([P, C], F32)
        r0ap = r0[0, :]
        nc.sync.dma_start(
            out=r0rep[:],
            in_=bass.AP(tensor=r0ap.tensor, offset=r0ap.offset,
                        ap=[[0, P]] + list(r0ap.ap)),
        )
        e0 = pool.tile([P, C], F32)
        nc.scalar.activation(out=e0[:], in_=r0rep[:], func=AF.Relu)
        nc.scalar.activation(out=e0[:], in_=e0[:], func=AF.Exp, scale=-1.0)
        e0ap = e0[:]

        gam = pool.tile([P, T, 1], F32)
        for j in range(n_chunks):
            sl = slice(j * TCH, (j + 1) * TCH)
            xt = ch.tile([P, TCH, C], F32, tag="xt")
            nc.sync.dma_start(out=xt[:], in_=xv[:, sl, :])
            f = ch.tile([P, TCH, C], F32, tag="f")
            nc.scalar.activation(out=f[:], in_=xt[:], func=AF.Relu)
            rmax = ch.tile([P, TCH, 1], F32, tag="rmax")
            nc.vector.reduce_max(out=rmax[:], in_=f[:], axis=mybir.AxisListType.X)
            ex = ch.tile([P, TCH, C], F32, tag="ex")
            nc.scalar.activation(out=ex[:], in_=f[:], func=AF.Exp)
            nc.vector.tensor_mul(ex[:], ex[:], f[:])
            e0b = bass.AP(tensor=e0ap.tensor, offset=e0ap.offset,
                          ap=[e0ap.ap[0], [0, TCH], e0ap.ap[1]])
            nc.vector.tensor_mul(ex[:], ex[:], e0b)
            m = ch.tile([P, TCH, 1], F32, tag="m")
            nc.vector.reduce_max(out=m[:], in_=ex[:], axis=mybir.AxisListType.X)
            rinv = ch.tile([P, TCH, 1], F32, tag="rinv")
            nc.vector.reciprocal(out=rinv[:], in_=rmax[:])
            nc.vector.tensor_mul(gam[:, sl, :], m[:], rinv[:])

        sq = pool.tile([P, T, 1], F32)
        ssq = pool.tile([P, 1], F32)
        nc.scalar.activation(out=sq[:], in_=gam[:], func=AF.Square,
                             accum_out=ssq[:])
        ones = pool.tile([P, 1], F32)
        nc.vector.memset(ones[:], 1.0)
        onesr = pool.tile([1, P], F32)
        nc.vector.memset(onesr[:], 1.0)
        tot = psum.tile([1, 1], F32)
        nc.tensor.matmul(tot[:], ssq[:], ones[:])
        tot_sb = pool.tile([1, 1], F32)
        nc.scalar.activation(out=tot_sb[:], in_=tot[:], func=AF.Copy)
        bc = psum.tile([P, 1], F32)
        nc.tensor.matmul(bc[:], onesr[:], tot_sb[:])
        rec = pool.tile([P, 1], F32)
        nc.vector.reciprocal(out=rec[:], in_=bc[:])
        rstd = pool.tile([P, 1], F32)
        nc.scalar.activation(out=rstd[:], in_=rec[:], func=AF.Sqrt)
        outt = pool.tile([P, T], F32)
        nc.scalar.activation(out=outt[:], in_=gam[:, :, 0], func=AF.Copy,
                             scale=rstd[:])
        nc.sync.dma_start(out=yv[:], in_=outt[:])

    nc.compile()
    return nc


_NC_CACHE = {}

IMPL = os.environ.get("KERNEL_IMPL", "v3")


def _parse_opts():
    """KERNEL_OPTS="n_chunks=6,bufs=2,use_f16=0" -> kwargs for build_v3."""
    opts = {}
    for kv in os.environ.get("KERNEL_OPTS", "").split(","):
        if not kv.strip():
            continue
        k, v = kv.split("=")
        if v in ("0", "1"):
            opts[k.strip()] = bool(int(v))
        elif v.isdigit():
            opts[k.strip()] = int(v)
        else:
            opts[k.strip()] = v.strip()
    return opts


def _get_nc():
    if "nc" not in _NC_CACHE:
        if IMPL == "v1":
            _NC_CACHE["nc"] = build_nc()
        else:
            _NC_CACHE["nc"] = build_v3(**_parse_opts())
    return _NC_CACHE["nc"]


def make_in_maps(feats):
    in_maps = []
    for core in range(NCORES):
        b = core if core < B else 0  # cores >= B chew duplicate data
        in_maps.append({
            "x": np.ascontiguousarray(feats[b]),
            "row0": np.ascontiguousarray(feats[b, 0:1, :]),
        })
    return in_maps


def gather_out(results):
    return np.concatenate([results[b]["y"] for b in range(B)])


def kernel(coords: np.ndarray, features: np.ndarray) -> np.ndarray:
    feats = np.ascontiguousarray(np.asarray(features), dtype=np.float32)
    assert feats.shape == (B, N, C), feats.shape
    nc = _get_nc()
    res = run_bass_kernel_spmd(nc, make_in_maps(feats),
                               core_ids=list(range(NCORES)))
    return gather_out(res.results).astype(np.float32)
